# revision 1
# baseline (speedup 1.0000x reference)
"""ANFIS forward kernel for Trainium2 (8 NeuronCores, data-parallel over batch).

Problem shapes (hardcoded): B=16384, R=512 rules, F=32 features, O=8 outputs.

Math (identical to the reference, re-associated for the PE):
  a[r]            = -1 / (2*(|w[r]|+0.1)^2)
  v[r,b]          = a[r] * (x2[b] + c2[r] - 2*cross[b,r])      (= -dist/(2 s^2))
  f[r,b]          = exp(v[r,b])                                 (UNnormalized)
  H[b, o*32+f]    = sum_r f[r,b] * W[r,f,o]
  H[b, 256+o]     = sum_r f[r,b] * cb[r,o]
  H[b, 264]       = sum_r f[r,b]                  (= S[b])
  out[b,o]        = (sum_f x[b,f]*H[b,o*32+f] + H[b,256+o]) / (S[b]+1e-8)

Device mapping per core (2048 batch rows, 4 groups of 512):
  MM1 (PE):  psum[128r, 512b] = daug[:, rc]^T @ xat_group, computed as a
             3-pass fp16 hi/lo split (Dh@Xh + Dh@Xl + Dl@Xh accumulated in
             fp32 PSUM; dropped Dl@Xl term is ~2^-22 relative).  At the fixed
             1.2 GHz PE clock of this part, fp16 streams 1 cycle/row vs 4 for
             fp32, so 3 passes beat one fp32 matmul by ~25%.  Firing comes out
             TRANSPOSED (rules on partitions) - exactly the layout MM2 needs.
  EXP (ACT): firing tile = exp(psum)
  MM2 (PE):  psum_h[128b, jslot, 266] += firing[rc][:, j128]^T @ wr2[rc]
             in float32r (1 cycle/row at N>=256; dst padded to even count).
  Combine (DVE), once per 512-row group over a 4-bank psum tile:
             broadcast mul (x repeated over o via 0-stride AP) + reduce_sum
             + bias add, then out = numerator * reciprocal(S + 1e-8).

Notes:
  * Built on bacc.Bacc + nc.compile(): generate_event_semaphores legalizes the
    1-wait-per-instruction TRN2 limit, so multi-dep matmuls/DMAs are fine.
  * tensor_tensor_reduce hangs this hardware/runtime combination - do not use.
  * Inputs are loaded in per-group chunks so DMAs spread over the 8 HWDGE
    queues and the first MM1 only waits for its own ~100 KB, not ~1 MB.
"""

import numpy as np

import concourse.bacc as bacc
import concourse.bass as bass
import concourse.tile as tile
from concourse import mybir
from concourse.bass_utils import run_bass_kernel_spmd

B, R, F, O = 16384, 512, 32, 8
NCORES = 8
BL = B // NCORES           # 2048 batch rows per core
GW = 512                   # batch-group width for MM1 streaming
NG = BL // GW              # 4 groups per core
NT = BL // 128             # 16 b128 tiles per core
JG = GW // 128             # 4 b128 tiles per group
K1 = F + 2                 # 34 = x rows + ones row + x2 row
NC2 = O * F + O + 2        # 266 = (o,f) block + bias + ones + pad (fp32r even dst)
RC = R // 128              # 4 rule chunks
SCOL = O * F + O           # 264: column holding S = sum_r firing

F32 = mybir.dt.float32
F16 = mybir.dt.float16
F32R = mybir.dt.float32r
EXP = mybir.ActivationFunctionType.Exp

_PROGRAM = None


def _build_program():
    nc = bacc.Bacc()
    xah_d = nc.declare_dram_parameter("xah", [K1, BL], F16, isOutput=False)
    xal_d = nc.declare_dram_parameter("xal", [K1, BL], F16, isOutput=False)
    dh_d = nc.declare_dram_parameter("dh", [K1, R], F16, isOutput=False)
    dl_d = nc.declare_dram_parameter("dl", [K1, R], F16, isOutput=False)
    x_d = nc.declare_dram_parameter("x", [BL, F], F32, isOutput=False)
    wr2_d = nc.declare_dram_parameter("wr2", [RC, 128, NC2], F32R, isOutput=False)
    out_d = nc.declare_dram_parameter("out", [BL, O], F32, isOutput=True)

    with tile.TileContext(nc) as tc:
        with (
            tc.tile_pool(name="one", bufs=1) as one,
            tc.tile_pool(name="xa", bufs=2 * NG) as xa_pool,
            tc.tile_pool(name="xt", bufs=2) as xt_pool,
            tc.tile_pool(name="ft", bufs=2 * RC) as ft_pool,
            tc.tile_pool(name="scr", bufs=2) as scr_pool,
            tc.tile_pool(name="sm", bufs=3) as sm_pool,
            tc.tile_pool(name="ot", bufs=2) as ot_pool,
            tc.tile_pool(name="ps1", bufs=2, space="PSUM") as ps1_pool,
            tc.tile_pool(name="psh", bufs=1, space="PSUM") as psh_pool,
        ):
            # group-0 MM1 operands first so the PE can start ASAP; the DMA
            # queue round-robin spreads these over distinct HWDGE queues.
            xa_h = [
                xa_pool.tile([K1, GW], F16, tag="xah", name=f"xah{g}")
                for g in range(NG)
            ]
            xa_l = [
                xa_pool.tile([K1, GW], F16, tag="xal", name=f"xal{g}")
                for g in range(NG)
            ]
            nc.sync.dma_start(out=xa_h[0][:], in_=xah_d[:, 0:GW])
            nc.sync.dma_start(out=xa_l[0][:], in_=xal_d[:, 0:GW])
            dh_sb = one.tile([K1, R], F16, tag="dh")
            nc.sync.dma_start(out=dh_sb[:], in_=dh_d[:])
            dl_sb = one.tile([K1, R], F16, tag="dl")
            nc.sync.dma_start(out=dl_sb[:], in_=dl_d[:])
            w_sb = one.tile([128, RC, NC2], F32R, tag="wr2")
            for rc in range(RC):
                nc.sync.dma_start(out=w_sb[:, rc, :], in_=wr2_d[rc])
            for g in range(1, NG):
                nc.sync.dma_start(out=xa_h[g][:], in_=xah_d[:, g * GW:(g + 1) * GW])
                nc.sync.dma_start(out=xa_l[g][:], in_=xal_d[:, g * GW:(g + 1) * GW])
            x_g = []
            for g in range(NG):
                xg = xt_pool.tile([128, JG, F], F32, tag=f"xg{g % 2}")
                nc.sync.dma_start(
                    out=xg[:],
                    in_=x_d[g * GW:(g + 1) * GW].rearrange("(j p) f -> p j f", p=128),
                )
                x_g.append(xg)

            for g in range(NG):
                fts = []
                for rc in range(RC):
                    ps1 = ps1_pool.tile([128, GW], F32, tag="ps1")
                    rsl = slice(rc * 128, (rc + 1) * 128)
                    nc.tensor.matmul(
                        ps1[:], dh_sb[:, rsl], xa_h[g][:], start=True, stop=False
                    )
                    nc.tensor.matmul(
                        ps1[:], dh_sb[:, rsl], xa_l[g][:], start=False, stop=False
                    )
                    nc.tensor.matmul(
                        ps1[:], dl_sb[:, rsl], xa_h[g][:], start=False, stop=True
                    )
                    ft = ft_pool.tile([128, GW], F32R, tag="ft")
                    nc.scalar.activation(ft[:], ps1[:], EXP)
                    fts.append(ft)

                # MM2 for the whole group into one 4-bank psum tile
                psh = psh_pool.tile([128, JG, 512], F32, tag="psh")
                for j in range(JG):
                    for rc in range(RC):
                        nc.tensor.matmul(
                            psh[:, j, 0:NC2],
                            fts[rc][:, j * 128:(j + 1) * 128],
                            w_sb[:, rc, :],
                            start=(rc == 0),
                            stop=(rc == RC - 1),
                        )

                # ---- combine for the whole group on DVE ----
                xa = x_g[g][:]                       # [128, JG, F]
                xb = bass.AP(
                    tensor=xa.tensor,
                    offset=xa.offset,
                    ap=[xa.ap[0], xa.ap[1], [0, O], xa.ap[2]],
                )
                scratch = scr_pool.tile([128, JG, O, F], F32, tag="scr")
                nc.vector.tensor_mul(
                    scratch[:],
                    psh[:, :, 0:O * F].rearrange("p j (o f) -> p j o f", o=O),
                    xb,
                )
                osum = sm_pool.tile([128, JG, O], F32, tag="osum")
                nc.vector.reduce_sum(
                    out=osum[:], in_=scratch[:], axis=mybir.AxisListType.X
                )
                num_t = sm_pool.tile([128, JG, O], F32, tag="num")
                nc.vector.tensor_add(num_t[:], osum[:], psh[:, :, O * F:SCOL])

                sden = sm_pool.tile([128, JG, 1], F32, tag="sden")
                nc.vector.tensor_scalar_add(sden[:], psh[:, :, SCOL:SCOL + 1], 1e-8)
                rec = sm_pool.tile([128, JG, 1], F32, tag="rec")
                nc.vector.reciprocal(rec[:], sden[:])
                ra = rec[:]
                rb = bass.AP(
                    tensor=ra.tensor, offset=ra.offset, ap=[ra.ap[0], ra.ap[1], [0, O]]
                )
                out_t = ot_pool.tile([128, JG, O], F32, tag="outt")
                nc.vector.tensor_mul(out_t[:], num_t[:], rb)
                nc.sync.dma_start(
                    out=out_d[g * GW:(g + 1) * GW].rearrange("(j p) o -> p j o", p=128),
                    in_=out_t[:],
                )
    nc.compile()
    return nc


def get_program():
    global _PROGRAM
    if _PROGRAM is None:
        _PROGRAM = _build_program()
    return _PROGRAM


def _split_f16(arr):
    hi = arr.astype(np.float16)
    lo = (arr - hi.astype(np.float32)).astype(np.float16)
    return np.ascontiguousarray(hi), np.ascontiguousarray(lo)


def make_in_maps(inputs, centers, widths, consequent_w, consequent_b):
    x = np.ascontiguousarray(np.asarray(inputs, dtype=np.float32))
    c64 = np.asarray(centers, dtype=np.float64)
    w64 = np.asarray(widths, dtype=np.float64)
    W = np.asarray(consequent_w, dtype=np.float32)
    cb = np.asarray(consequent_b, dtype=np.float32)

    s = np.abs(w64) + 0.1
    a = -1.0 / (2.0 * s * s)                       # [R]

    daug = np.empty((K1, R), dtype=np.float64)
    daug[:F] = (c64 * (-2.0 * a)[:, None]).T       # -2*c[r,f]*a[r]
    daug[F] = (c64 * c64).sum(axis=1) * a          # c2[r]*a[r]  (pairs with ones)
    daug[F + 1] = a                                # a[r]        (pairs with x2)
    dh, dl = _split_f16(daug.astype(np.float32))

    wr2 = np.zeros((R, NC2), dtype=np.float32)
    wr2[:, :F * O] = W.transpose(0, 2, 1).reshape(R, F * O)   # col = o*32+f
    wr2[:, F * O:F * O + O] = cb
    wr2[:, SCOL] = 1.0                                        # S column; last col 0
    wr2 = np.ascontiguousarray(wr2.reshape(RC, 128, NC2))

    x2 = np.einsum("bf,bf->b", x, x).astype(np.float32)
    xat = np.empty((K1, B), dtype=np.float32)
    xat[:F] = x.T
    xat[F] = 1.0
    xat[F + 1] = x2
    xah, xal = _split_f16(xat)

    in_maps = []
    for ci in range(NCORES):
        sl = slice(ci * BL, (ci + 1) * BL)
        in_maps.append({
            "xah": np.ascontiguousarray(xah[:, sl]),
            "xal": np.ascontiguousarray(xal[:, sl]),
            "dh": dh,
            "dl": dl,
            "x": np.ascontiguousarray(x[sl]),
            "wr2": wr2,
        })
    return in_maps


def _axon_reset():
    """Recover a wedged NeuronCore (NRT_EXEC_UNIT_UNRECOVERABLE) via the axon
    client's reset entry point.  Best-effort: silently skipped off-axon."""
    try:
        import ctypes
        import time

        import jax

        jax.devices()
        lib = ctypes.CDLL("/opt/axon/libaxon_pjrt.so")
        lib.axon_reset.restype = ctypes.c_int64
        lib.axon_reset()
        time.sleep(2)
    except Exception:
        pass


def kernel(inputs, centers, widths, consequent_w, consequent_b):
    nc = get_program()
    in_maps = make_in_maps(inputs, centers, widths, consequent_w, consequent_b)
    last_err = None
    for attempt in range(3):
        try:
            res = run_bass_kernel_spmd(nc, in_maps, list(range(NCORES))).results
            return np.concatenate([r["out"] for r in res], axis=0)
        except Exception as e:  # wedged device -> reset + retry
            last_err = e
            _axon_reset()
    raise last_err



# revision 3
# speedup vs baseline: 1.4088x; 1.4088x over previous
"""ANFIS forward kernel for Trainium2 (8 NeuronCores, data-parallel over batch).

Problem shapes (hardcoded): B=16384, R=512 rules, F=32 features, O=8 outputs.

Math (identical to the reference, re-associated for the PE):
  a[r]            = -1 / (2*(|w[r]|+0.1)^2)
  v[r,b]          = a[r] * (x2[b] + c2[r] - 2*cross[b,r])      (= -dist/(2 s^2))
  f[r,b]          = exp(v[r,b])                                 (UNnormalized)
  H[b, o*33+f]    = sum_r f[r,b] * Waug[r,f,o]   (f=32 slot holds the bias)
  H[b, 264]       = sum_r f[r,b]                  (= S[b])
  out[b,o]        = (sum_f xaug[b,f]*H[b,o*33+f]) / (S[b]+1e-8)

Device mapping per core (2048 batch rows, 4 groups of 512):
  MM1 (PE):  psum[128r, 512b] = dpack[:, rc]^T @ xpack_g, ONE fp16 matmul with
             the hi/lo split packed along the contraction dim:
             K=102 rows = [Dh;Dh;Dl] against [Xh;Xl;Xh], so the three partial
             products of the fp16 pair accumulate inside a single pass
             (dropped Dl@Xl term is ~2^-22 relative).  512 streamed rows
             @1 row/cycle vs 3x512 for the 3-pass variant.
  EXP (ACT): firing tile = exp(psum), fp32 (firing spans e^-13..e^-128;
             16-bit floats would flush/degrade columns whose S ~ 1e-8).
  MM2 (PE):  psum_h[128b, 266] += firing[rc][:, j128]^T @ wt[rc] in float32r
             (1 cycle/row at N>=256; dst padded to even count).  Software
             pipelined one group behind MM1 so all four exp tiles of a group
             are ready before its MM2 starts (no PE stall on ACT latency).
  Combine (DVE), per 128-row j-chunk over a 1-bank psum tile:
             broadcast mul (xaug repeated over o via 0-stride AP) +
             reduce_sum, then out = numerator * reciprocal(S + 1e-8).

DMA notes (this part's HWDGE):
  * dma_start issue costs ~650ns serialized on the issuing queue; descriptors
    round-robin over all 16 DMA engines in chunks of 32.  So: FEW dma_starts,
    each with MANY ~1KB descriptors (max_dma_last_dim forces the split).
  * All host-side arrays are pre-laid-out so every DMA is a contiguous
    partition-major block: xd = [102, 5, 512] fp16 (4 x-groups + dpack in one
    transfer), wt = [128, RC, 266] f32, xc = [128, NG, 132] f32,
    out = [128, NG, JG*O] f32 (host inverse-permutes after gather).
  * Issue split across the two HWDGE queues (sync: xd/xc/out, scalar: wt).

Notes:
  * Built on bacc.Bacc + nc.compile(): generate_event_semaphores legalizes the
    1-wait-per-instruction TRN2 limit, so multi-dep matmuls/DMAs are fine.
  * tensor_tensor_reduce hangs this hardware/runtime combination - do not use.
"""

import numpy as np

import concourse.bacc as bacc
import concourse.bass as bass
import concourse.tile as tile
from concourse import mybir
from concourse.bass_utils import run_bass_kernel_spmd

B, R, F, O = 16384, 512, 32, 8
NCORES = 8
BL = B // NCORES           # 2048 batch rows per core
GW = 512                   # batch-group width for MM1 streaming
NG = BL // GW              # 4 groups per core
JG = GW // 128             # 4 b128 tiles per group
K1 = F + 2                 # 34 = x rows + ones row + x2 row
KP = 3 * K1                # 102 = packed contraction dim [h;l;h]
F1 = F + 1                 # 33 = x features + ones (bias slot)
NC2 = O * F1 + 2           # 266 = (o,f+bias) block + S + pad (fp32r even dst)
RC = R // 128              # 4 rule chunks
SCOL = O * F1              # 264: column holding S = sum_r firing

F32 = mybir.dt.float32
F16 = mybir.dt.float16
F32R = mybir.dt.float32r
EXP = mybir.ActivationFunctionType.Exp

_PROGRAM = None


def _build_program():
    nc = bacc.Bacc()
    xd_d = nc.declare_dram_parameter("xd", [KP, NG + 1, GW], F16, isOutput=False)
    wt_d = nc.declare_dram_parameter("wt", [128, RC, NC2], F32R, isOutput=False)
    xc_d = nc.declare_dram_parameter("xc", [128, NG, JG * F1], F32, isOutput=False)
    out_d = nc.declare_dram_parameter("out", [128, NG, JG * O], F32, isOutput=True)

    with tile.TileContext(nc) as tc:
        with (
            tc.tile_pool(name="one", bufs=1) as one,
            tc.tile_pool(name="ft", bufs=10) as ft_pool,
            tc.tile_pool(name="scr", bufs=3) as scr_pool,
            tc.tile_pool(name="sm", bufs=3) as sm_pool,
            tc.tile_pool(name="ot", bufs=2) as ot_pool,
            tc.tile_pool(name="ps1", bufs=3, space="PSUM") as ps1_pool,
            tc.tile_pool(name="psh", bufs=4, space="PSUM") as psh_pool,
        ):
            # xd carries the 4 batch groups AND dpack (slot NG) in one DMA:
            # 510 1KB descriptors spread over all 16 DMA engines.
            xd_sb = one.tile([KP, NG + 1, GW], F16, tag="xd")
            nc.sync.dma_start(out=xd_sb[:], in_=xd_d[:], max_dma_last_dim=GW)
            w_sb = one.tile([128, RC, NC2], F32R, tag="wt")
            nc.scalar.dma_start(out=w_sb[:], in_=wt_d[:], max_dma_last_dim=NC2)
            xc_sb = one.tile([128, NG, JG, F1], F32, tag="xc")
            nc.sync.dma_start(
                out=xc_sb[:], in_=xc_d[:], max_dma_last_dim=JG * F1
            )

            fts = [[None] * RC for _ in range(NG)]

            def emit_mm2_combine(g):
                out_t = ot_pool.tile([128, JG, O], F32, tag="outt")
                for j in range(JG):
                    psh = psh_pool.tile([128, GW], F32, tag="psh")
                    for rc in range(RC):
                        nc.tensor.matmul(
                            psh[:, 0:NC2],
                            fts[g][rc][:, j * 128:(j + 1) * 128],
                            w_sb[:, rc, :],
                            start=(rc == 0),
                            stop=(rc == RC - 1),
                        )
                    # ---- combine for this j-chunk on DVE ----
                    xv = xc_sb[:, g, j, :]           # [128, F1]
                    xb = bass.AP(
                        tensor=xv.tensor,
                        offset=xv.offset,
                        ap=[xv.ap[0], [0, O], xv.ap[1]],
                    )
                    scratch = scr_pool.tile([128, O, F1], F32, tag="scr")
                    nc.vector.tensor_mul(
                        scratch[:],
                        psh[:, 0:SCOL].rearrange("p (o f) -> p o f", o=O),
                        xb,
                    )
                    osum = sm_pool.tile([128, O], F32, tag="osum")
                    nc.vector.reduce_sum(
                        out=osum[:], in_=scratch[:], axis=mybir.AxisListType.X
                    )
                    sden = sm_pool.tile([128, 1], F32, tag="sden")
                    nc.vector.tensor_scalar_add(
                        sden[:], psh[:, SCOL:SCOL + 1], 1e-8
                    )
                    rec = sm_pool.tile([128, 1], F32, tag="rec")
                    nc.vector.reciprocal(rec[:], sden[:])
                    ra = rec[:]
                    rb = bass.AP(
                        tensor=ra.tensor, offset=ra.offset, ap=[ra.ap[0], [0, O]]
                    )
                    nc.vector.tensor_mul(out_t[:, j, :], osum[:], rb)
                nc.sync.dma_start(out=out_d[:, g], in_=out_t[:])

            for g in range(NG):
                for rc in range(RC):
                    ps1 = ps1_pool.tile([128, GW], F32, tag="ps1")
                    nc.tensor.matmul(
                        ps1[:],
                        xd_sb[:, NG, rc * 128:(rc + 1) * 128],
                        xd_sb[:, g, :],
                        start=True,
                        stop=True,
                    )
                    ft = ft_pool.tile([128, GW], F32R, tag="ft")
                    nc.scalar.activation(ft[:], ps1[:], EXP)
                    fts[g][rc] = ft
                # MM2 one group behind: its 4 exp tiles are already done.
                if g > 0:
                    emit_mm2_combine(g - 1)
            emit_mm2_combine(NG - 1)
    nc.compile()
    return nc


def get_program():
    global _PROGRAM
    if _PROGRAM is None:
        _PROGRAM = _build_program()
    return _PROGRAM


def _split_f16(arr):
    hi = arr.astype(np.float16)
    lo = (arr - hi.astype(np.float32)).astype(np.float16)
    return np.ascontiguousarray(hi), np.ascontiguousarray(lo)


def make_in_maps(inputs, centers, widths, consequent_w, consequent_b):
    x = np.ascontiguousarray(np.asarray(inputs, dtype=np.float32))
    c64 = np.asarray(centers, dtype=np.float64)
    w64 = np.asarray(widths, dtype=np.float64)
    W = np.asarray(consequent_w, dtype=np.float32)
    cb = np.asarray(consequent_b, dtype=np.float32)

    s = np.abs(w64) + 0.1
    a = -1.0 / (2.0 * s * s)                       # [R]

    daug = np.empty((K1, R), dtype=np.float64)
    daug[:F] = (c64 * (-2.0 * a)[:, None]).T       # -2*c[r,f]*a[r]
    daug[F] = (c64 * c64).sum(axis=1) * a          # c2[r]*a[r]  (pairs with ones)
    daug[F + 1] = a                                # a[r]        (pairs with x2)
    dh, dl = _split_f16(daug.astype(np.float32))
    dpack = np.concatenate([dh, dh, dl], axis=0)   # [102, R]

    # Waug columns: col o*33+f = W[r,f,o] (f<32), col o*33+32 = cb[r,o],
    # col 264 = 1.0 (S), col 265 = 0 (pad).
    wt = np.zeros((R, NC2), dtype=np.float32)
    for o in range(O):
        wt[:, o * F1:o * F1 + F] = W[:, :, o]
        wt[:, o * F1 + F] = cb[:, o]
    wt[:, SCOL] = 1.0
    wt = np.ascontiguousarray(wt.reshape(RC, 128, NC2).transpose(1, 0, 2))

    x2 = np.einsum("bf,bf->b", x, x).astype(np.float32)
    xat = np.empty((K1, B), dtype=np.float32)
    xat[:F] = x.T
    xat[F] = 1.0
    xat[F + 1] = x2
    xah, xal = _split_f16(xat)
    xpk = np.concatenate([xah, xal, xah], axis=0)  # [102, B]

    xaug = np.empty((B, F1), dtype=np.float32)
    xaug[:, :F] = x
    xaug[:, F] = 1.0

    in_maps = []
    for ci in range(NCORES):
        sl = slice(ci * BL, (ci + 1) * BL)
        xd = np.empty((KP, NG + 1, GW), dtype=np.float16)
        xd[:, :NG, :] = xpk[:, sl].reshape(KP, NG, GW)
        xd[:, NG, :] = dpack
        xc = np.ascontiguousarray(
            xaug[sl].reshape(NG, JG, 128, F1)
            .transpose(2, 0, 1, 3)
            .reshape(128, NG, JG * F1)
        )
        in_maps.append({
            "xd": np.ascontiguousarray(xd),
            "wt": wt,
            "xc": xc,
        })
    return in_maps


def assemble_out(results):
    """[128, NG, JG*O] per core -> full [B, O] (b = g*512 + j*128 + p)."""
    outs = []
    for r in results:
        o = np.asarray(r["out"]).reshape(128, NG, JG, O)
        outs.append(o.transpose(1, 2, 0, 3).reshape(BL, O))
    return np.concatenate(outs, axis=0)


def _axon_reset():
    """Recover a wedged NeuronCore (NRT_EXEC_UNIT_UNRECOVERABLE) via the axon
    client's reset entry point.  Best-effort: silently skipped off-axon."""
    try:
        import ctypes
        import time

        import jax

        jax.devices()
        lib = ctypes.CDLL("/opt/axon/libaxon_pjrt.so")
        lib.axon_reset.restype = ctypes.c_int64
        lib.axon_reset()
        time.sleep(2)
    except Exception:
        pass


def kernel(inputs, centers, widths, consequent_w, consequent_b):
    nc = get_program()
    in_maps = make_in_maps(inputs, centers, widths, consequent_w, consequent_b)
    last_err = None
    for attempt in range(3):
        try:
            res = run_bass_kernel_spmd(nc, in_maps, list(range(NCORES))).results
            return assemble_out(res)
        except Exception as e:  # wedged device -> reset + retry
            last_err = e
            _axon_reset()
    raise last_err


# revision 7
# speedup vs baseline: 1.4296x; 1.0148x over previous
"""ANFIS forward kernel for Trainium2 (8 NeuronCores, data-parallel over batch).

Problem shapes (hardcoded): B=16384, R=512 rules, F=32 features, O=8 outputs.

Math (identical to the reference, re-associated for the PE):
  a[r]            = -1 / (2*(|w[r]|+0.1)^2)
  v[r,b]          = a[r] * (x2[b] + c2[r] - 2*cross[b,r])      (= -dist/(2 s^2))
  f[r,b]          = exp(v[r,b])                                 (UNnormalized)
  H[b, o*33+f]    = sum_r f[r,b] * Waug[r,f,o]   (f=32 slot holds the bias)
  H[b, 264]       = sum_r f[r,b]                  (= S[b])
  out[b,o]        = (sum_f xaug[b,f]*H[b,o*33+f]) / (S[b]+1e-8)

Device mapping per core (2048 batch rows, 4 groups of 512):
  MM1 (PE):  psum[128r, 512b] = dpack[:, rc]^T @ xpack_g, ONE fp16 matmul with
             the hi/lo split packed along the contraction dim:
             K=102 rows = [Dh;Dh;Dl] against [Xh;Xl;Xh], so the three partial
             products of the fp16 pair accumulate inside a single pass
             (dropped Dl@Xl term is ~2^-22 relative).
  EXP (ACT): firing tile = exp(psum), fp32 (firing spans e^-13..e^-128;
             16-bit floats would flush/degrade columns whose S ~ 1e-8).
  MM2 (PE):  psum_h[128b, 266] += firing[rc][:, j128]^T @ wt[rc] in float32r
             (observed ~123ns issue rate at N=266).  Software pipelined one
             group behind MM1 so all four exp tiles of a group are ready
             before its MM2 starts (no PE stall on ACT latency).
  Combine, per j-PAIR over a 2-bank psum tile, split across DVE (pair 0)
             and GpSimd (pair 1) so neither engine gates the PE:
             broadcast mul (xaug repeated over o via 0-stride AP) +
             reduce_sum + eps-add, reciprocal on DVE (GpSimd lacks it),
             then out = numerator * rec.

DMA notes (this part's HWDGE):
  * dma_start issue costs ~650ns serialized on the issuing queue; descriptors
    round-robin over the 16 DMA engines in ~32-descriptor chunks, so a
    transfer only parallelizes if it has many (~1KB) descriptors.  DRAM-side
    rows are padded (512->520 etc.) so the AP optimizer cannot merge rows
    into multi-KB descriptors.
  * Issue split across the two HWDGE queues; the first transfer carries
    dpack + batch group 0 so MM1 starts as early as possible.
  * out = [128, NG, JG*O] f32 partition-major (host inverse-permutes after).

Notes:
  * Built on bacc.Bacc + nc.compile(): generate_event_semaphores legalizes the
    1-wait-per-instruction TRN2 limit, so multi-dep matmuls/DMAs are fine.
  * tensor_tensor_reduce hangs this hardware/runtime combination - do not use.
"""

import numpy as np

import concourse.bacc as bacc
import concourse.bass as bass
import concourse.tile as tile
from concourse import mybir
from concourse.bass_utils import run_bass_kernel_spmd

B, R, F, O = 16384, 512, 32, 8
NCORES = 8
BL = B // NCORES           # 2048 batch rows per core
GW = 512                   # batch-group width for MM1 streaming
GWP = 520                  # DRAM-padded group width (keeps descriptors at 1KB)
NG = BL // GW              # 4 groups per core
JG = GW // 128             # 4 b128 tiles per group
K1 = F + 2                 # 34 = x rows + ones row + x2 row
KP = 3 * K1                # 102 = packed contraction dim [h;l;h]
F1 = F + 1                 # 33 = x features + ones (bias slot)
NC2 = O * F1 + 2           # 266 = (o,f+bias) block + S + pad (fp32r even dst)
NC2P = 272                 # DRAM-padded wt row
RC = R // 128              # 4 rule chunks
SCOL = O * F1              # 264: column holding S = sum_r firing
XCW = JG * F1              # 132 xc row elems
XCWP = 136                 # DRAM-padded xc row

F32 = mybir.dt.float32
F16 = mybir.dt.float16
BF16 = mybir.dt.bfloat16
F32R = mybir.dt.float32r
EXP = mybir.ActivationFunctionType.Exp

_PROGRAM = None


def _build_program():
    nc = bacc.Bacc()
    xd_d = nc.declare_dram_parameter("xd", [KP, NG + 1, GWP], F16, isOutput=False)
    wt_d = nc.declare_dram_parameter("wt", [128, RC, NC2P], F32R, isOutput=False)
    xc_d = nc.declare_dram_parameter("xc", [128, NG, XCWP], F32, isOutput=False)
    out_d = nc.declare_dram_parameter("out", [128, NG, JG * O], F32, isOutput=True)

    with tile.TileContext(nc) as tc:
        with (
            tc.tile_pool(name="one", bufs=1) as one,
            tc.tile_pool(name="ft", bufs=10) as ft_pool,
            tc.tile_pool(name="scr", bufs=2) as scr_pool,
            tc.tile_pool(name="sm", bufs=2) as sm_pool,
            tc.tile_pool(name="ot", bufs=2) as ot_pool,
            tc.tile_pool(name="ps1", bufs=3, space="PSUM") as ps1_pool,
            tc.tile_pool(name="psh", bufs=2, space="PSUM") as psh_pool,
        ):
            # slot 0 = dpack, slots 1..4 = batch groups.  Two dma_starts so
            # the critical (dpack+g0) piece lands first; DRAM rows padded to
            # 520 so each descriptor stays one 512-elem (1KB) row.
            xd_sb = one.tile([KP, NG + 1, GW], F16, tag="xd")
            nc.sync.dma_start(out=xd_sb[:, 0:2], in_=xd_d[:, 0:2, 0:GW])
            nc.sync.dma_start(out=xd_sb[:, 2:], in_=xd_d[:, 2:, 0:GW])
            w_sb = one.tile([128, RC, NC2], F32R, tag="wt")
            nc.scalar.dma_start(out=w_sb[:], in_=wt_d[:, :, 0:NC2])
            xc_sb = one.tile([128, NG, JG, F1], F32, tag="xc")
            nc.scalar.dma_start(
                out=xc_sb[:],
                in_=xc_d[:, :, 0:XCW].rearrange("p g (j f) -> p g j f", f=F1),
            )

            fts = [[None] * RC for _ in range(NG)]

            def emit_pair(g, jj, osum_g, sden_g):
                """MM2 + numerator/denominator for j-chunks (2jj, 2jj+1)."""
                psh = psh_pool.tile([128, 2, GW], F32, tag="psh")
                for jp in range(2):
                    j = 2 * jj + jp
                    for rc in range(RC):
                        nc.tensor.matmul(
                            psh[:, jp, 0:NC2],
                            fts[g][rc][:, j * 128:(j + 1) * 128],
                            w_sb[:, rc, :],
                            start=(rc == 0),
                            stop=(rc == RC - 1),
                        )
                # GPSIMD cannot read PSUM, so everything touching psh is DVE.
                # bf16 scratch halves the reduce's input cost.
                xv = xc_sb[:, g, 2 * jj:2 * jj + 2, :]     # [128, 2, F1]
                xb = bass.AP(
                    tensor=xv.tensor,
                    offset=xv.offset,
                    ap=[xv.ap[0], xv.ap[1], [0, O], xv.ap[2]],
                )
                scratch = scr_pool.tile([128, 2, O, F1], BF16, tag="scr")
                nc.vector.tensor_mul(
                    scratch[:],
                    psh[:, :, 0:SCOL].rearrange("p j (o f) -> p j o f", o=O),
                    xb,
                )
                nc.vector.reduce_sum(
                    out=osum_g[:, jj], in_=scratch[:], axis=mybir.AxisListType.X
                )
                nc.vector.tensor_scalar_add(
                    sden_g[:, jj], psh[:, :, SCOL:SCOL + 1], 1e-8
                )

            def emit_mm2_combine(g):
                out_t = ot_pool.tile([128, JG, O], F32, tag="outt")
                osum_g = sm_pool.tile([128, 2, 2, O], F32, tag="osum")
                sden_g = sm_pool.tile([128, 2, 2, 1], F32, tag="sden")
                emit_pair(g, 0, osum_g, sden_g)
                emit_pair(g, 1, osum_g, sden_g)
                rec = sm_pool.tile([128, 2, 2, 1], F32, tag="rec")
                nc.vector.reciprocal(rec[:], sden_g[:])
                ra = rec[:]
                rb = bass.AP(
                    tensor=ra.tensor,
                    offset=ra.offset,
                    ap=[ra.ap[0], ra.ap[1], ra.ap[2], [0, O]],
                )
                # Final scale is SBUF-only, so GpSimd takes it off DVE.
                nc.gpsimd.tensor_mul(
                    out_t[:].rearrange("p (h t) o -> p h t o", h=2), osum_g[:], rb
                )
                nc.sync.dma_start(out=out_d[:, g], in_=out_t[:])

            for g in range(NG):
                for rc in range(RC):
                    ps1 = ps1_pool.tile([128, GW], F32, tag="ps1")
                    nc.tensor.matmul(
                        ps1[:],
                        xd_sb[:, 0, rc * 128:(rc + 1) * 128],
                        xd_sb[:, 1 + g, :],
                        start=True,
                        stop=True,
                    )
                    ft = ft_pool.tile([128, GW], F32R, tag="ft")
                    nc.scalar.activation(ft[:], ps1[:], EXP)
                    fts[g][rc] = ft
                # MM2 one group behind: its 4 exp tiles are already done.
                if g > 0:
                    emit_mm2_combine(g - 1)
            emit_mm2_combine(NG - 1)
    nc.compile()
    return nc


def get_program():
    global _PROGRAM
    if _PROGRAM is None:
        _PROGRAM = _build_program()
    return _PROGRAM


def _split_f16(arr):
    hi = arr.astype(np.float16)
    lo = (arr - hi.astype(np.float32)).astype(np.float16)
    return np.ascontiguousarray(hi), np.ascontiguousarray(lo)


def make_in_maps(inputs, centers, widths, consequent_w, consequent_b):
    x = np.ascontiguousarray(np.asarray(inputs, dtype=np.float32))
    c64 = np.asarray(centers, dtype=np.float64)
    w64 = np.asarray(widths, dtype=np.float64)
    W = np.asarray(consequent_w, dtype=np.float32)
    cb = np.asarray(consequent_b, dtype=np.float32)

    s = np.abs(w64) + 0.1
    a = -1.0 / (2.0 * s * s)                       # [R]

    daug = np.empty((K1, R), dtype=np.float64)
    daug[:F] = (c64 * (-2.0 * a)[:, None]).T       # -2*c[r,f]*a[r]
    daug[F] = (c64 * c64).sum(axis=1) * a          # c2[r]*a[r]  (pairs with ones)
    daug[F + 1] = a                                # a[r]        (pairs with x2)
    dh, dl = _split_f16(daug.astype(np.float32))
    dpack = np.concatenate([dh, dh, dl], axis=0)   # [102, R]

    # Waug columns: col o*33+f = W[r,f,o] (f<32), col o*33+32 = cb[r,o],
    # col 264 = 1.0 (S), cols 265.. = 0 (pad).
    wt = np.zeros((R, NC2P), dtype=np.float32)
    for o in range(O):
        wt[:, o * F1:o * F1 + F] = W[:, :, o]
        wt[:, o * F1 + F] = cb[:, o]
    wt[:, SCOL] = 1.0
    wt = np.ascontiguousarray(wt.reshape(RC, 128, NC2P).transpose(1, 0, 2))

    x2 = np.einsum("bf,bf->b", x, x).astype(np.float32)
    xat = np.empty((K1, B), dtype=np.float32)
    xat[:F] = x.T
    xat[F] = 1.0
    xat[F + 1] = x2
    xah, xal = _split_f16(xat)
    xpk = np.concatenate([xah, xal, xah], axis=0)  # [102, B]

    xaug = np.empty((B, F1), dtype=np.float32)
    xaug[:, :F] = x
    xaug[:, F] = 1.0

    in_maps = []
    for ci in range(NCORES):
        sl = slice(ci * BL, (ci + 1) * BL)
        xd = np.zeros((KP, NG + 1, GWP), dtype=np.float16)
        xd[:, 0, :GW] = dpack
        xd[:, 1:, :GW] = xpk[:, sl].reshape(KP, NG, GW)
        xc = np.zeros((128, NG, XCWP), dtype=np.float32)
        xc[:, :, :XCW] = (
            xaug[sl].reshape(NG, JG, 128, F1)
            .transpose(2, 0, 1, 3)
            .reshape(128, NG, XCW)
        )
        in_maps.append({
            "xd": np.ascontiguousarray(xd),
            "wt": wt,
            "xc": np.ascontiguousarray(xc),
        })
    return in_maps


def assemble_out(results):
    """[128, NG, JG*O] per core -> full [B, O] (b = g*512 + j*128 + p)."""
    outs = []
    for r in results:
        o = np.asarray(r["out"]).reshape(128, NG, JG, O)
        outs.append(o.transpose(1, 2, 0, 3).reshape(BL, O))
    return np.concatenate(outs, axis=0)


def _axon_reset():
    """Recover a wedged NeuronCore (NRT_EXEC_UNIT_UNRECOVERABLE) via the axon
    client's reset entry point.  Best-effort: silently skipped off-axon."""
    try:
        import ctypes
        import time

        import jax

        jax.devices()
        lib = ctypes.CDLL("/opt/axon/libaxon_pjrt.so")
        lib.axon_reset.restype = ctypes.c_int64
        lib.axon_reset()
        time.sleep(2)
    except Exception:
        pass


def kernel(inputs, centers, widths, consequent_w, consequent_b):
    nc = get_program()
    in_maps = make_in_maps(inputs, centers, widths, consequent_w, consequent_b)
    last_err = None
    for attempt in range(3):
        try:
            res = run_bass_kernel_spmd(nc, in_maps, list(range(NCORES))).results
            return assemble_out(res)
        except Exception as e:  # wedged device -> reset + retry
            last_err = e
            _axon_reset()
    raise last_err


# revision 9
# speedup vs baseline: 1.5418x; 1.0785x over previous
"""ANFIS forward kernel for Trainium2 (8 NeuronCores, data-parallel over batch).

Problem shapes (hardcoded): B=16384, R=512 rules, F=32 features, O=8 outputs.

Math (identical to the reference, re-associated for the PE):
  a[r]            = -1 / (2*(|w[r]|+0.1)^2)
  v[r,b]          = a[r] * (x2[b] + c2[r] - 2*cross[b,r])      (= -dist/(2 s^2))
  f[r,b]          = exp(v[r,b])                                 (UNnormalized)
  H[b, o*33+f]    = sum_r f[r,b] * Waug[r,f,o]   (f=32 slot holds the bias)
  H[b, 264]       = sum_r f[r,b]                  (= S[b])
  out[b,o]        = (sum_f xaug[b,f]*H[b,o*33+f]) / (S[b]+1e-8)

Device mapping per core (2048 batch rows, 4 groups of 512):
  MM1 (PE):  psum[128r, 512b] = dpack[:, rc]^T @ xpack_g, ONE fp16 matmul with
             the hi/lo split packed along the contraction dim:
             K=102 rows = [Dh;Dh;Dl] against [Xh;Xl;Xh], so the three partial
             products of the fp16 pair accumulate inside a single pass
             (dropped Dl@Xl term is ~2^-22 relative).
  EXP (ACT): firing tile = exp(psum), fp32 (firing spans e^-13..e^-128;
             16-bit floats would flush/degrade columns whose S ~ 1e-8).
  MM2 (PE):  psum_h[128b, 266] += firing[rc][:, j128]^T @ wt[rc] in float32r
             (observed ~123ns issue rate at N=266).  Software pipelined one
             group behind MM1 so all four exp tiles of a group are ready
             before its MM2 starts (no PE stall on ACT latency).
  Combine, per j-PAIR over a 2-bank psum tile, split across DVE (pair 0)
             and GpSimd (pair 1) so neither engine gates the PE:
             broadcast mul (xaug repeated over o via 0-stride AP) +
             reduce_sum + eps-add, reciprocal on DVE (GpSimd lacks it),
             then out = numerator * rec.

DMA notes (this part's HWDGE):
  * dma_start issue costs ~650ns serialized on the issuing queue; descriptors
    round-robin over the 16 DMA engines in ~32-descriptor chunks, so a
    transfer only parallelizes if it has many (~1KB) descriptors.  DRAM-side
    rows are padded (512->520 etc.) so the AP optimizer cannot merge rows
    into multi-KB descriptors.
  * Issue split across the two HWDGE queues; the first transfer carries
    dpack + batch group 0 so MM1 starts as early as possible.
  * out = [128, NG, JG*O] f32 partition-major (host inverse-permutes after).

Notes:
  * Built on bacc.Bacc + nc.compile(): generate_event_semaphores legalizes the
    1-wait-per-instruction TRN2 limit, so multi-dep matmuls/DMAs are fine.
  * tensor_tensor_reduce hangs this hardware/runtime combination - do not use.
"""

import numpy as np

import concourse.bacc as bacc
import concourse.bass as bass
import concourse.tile as tile
from concourse import mybir
from concourse.bass_utils import run_bass_kernel_spmd

B, R, F, O = 16384, 512, 32, 8
NCORES = 8
BL = B // NCORES           # 2048 batch rows per core
GW = 512                   # batch-group width for MM1 streaming
GWP = 520                  # DRAM-padded group width (keeps descriptors at 1KB)
NG = BL // GW              # 4 groups per core
JG = GW // 128             # 4 b128 tiles per group
K1 = F + 2                 # 34 = x rows + ones row + x2 row
KP = 3 * K1                # 102 = packed contraction dim [h;l;h]
F1 = F + 1                 # 33 = x features + ones (bias slot)
NC2 = O * F1 + 2           # 266 = (o,f+bias) block + S + pad (fp32r even dst)
NC2P = 272                 # DRAM-padded wt row
RC = R // 128              # 4 rule chunks
SCOL = O * F1              # 264: column holding S = sum_r firing
XCW = JG * F1              # 132 xc row elems
XCWP = 136                 # DRAM-padded xc row

F32 = mybir.dt.float32
F16 = mybir.dt.float16
BF16 = mybir.dt.bfloat16
F32R = mybir.dt.float32r
EXP = mybir.ActivationFunctionType.Exp

_PROGRAM = None


def _build_program():
    nc = bacc.Bacc()
    xd_d = nc.declare_dram_parameter("xd", [KP, NG + 1, GWP], F16, isOutput=False)
    wt_d = nc.declare_dram_parameter("wt", [128, RC, NC2P], F32R, isOutput=False)
    xc_d = nc.declare_dram_parameter("xc", [128, NG, XCWP], F32, isOutput=False)
    out_d = nc.declare_dram_parameter("out", [128, NG, JG * O], F32, isOutput=True)

    with tile.TileContext(nc) as tc:
        with (
            tc.tile_pool(name="one", bufs=1) as one,
            tc.tile_pool(name="ft", bufs=10) as ft_pool,
            tc.tile_pool(name="scr", bufs=2) as scr_pool,
            tc.tile_pool(name="sm", bufs=2) as sm_pool,
            tc.tile_pool(name="ot", bufs=2) as ot_pool,
            tc.tile_pool(name="ps1", bufs=2, space="PSUM") as ps1_pool,
            tc.tile_pool(name="psh", bufs=3, space="PSUM") as psh_pool,
        ):
            # slot 0 = dpack, slots 1..4 = batch groups.  Two dma_starts so
            # the critical (dpack+g0) piece lands first; DRAM rows padded to
            # 520 so each descriptor stays one 512-elem (1KB) row.
            xd_sb = one.tile([KP, NG + 1, GW], F16, tag="xd")
            nc.sync.dma_start(out=xd_sb[:, 0:2], in_=xd_d[:, 0:2, 0:GW])
            w_sb = one.tile([128, RC, NC2], F32R, tag="wt")
            nc.scalar.dma_start(out=w_sb[:], in_=wt_d[:, :, 0:NC2])
            nc.scalar.dma_start(out=xd_sb[:, 2:], in_=xd_d[:, 2:, 0:GW])
            xc_sb = one.tile([128, NG, JG, F1], F32, tag="xc")
            nc.scalar.dma_start(
                out=xc_sb[:],
                in_=xc_d[:, :, 0:XCW].rearrange("p g (j f) -> p g j f", f=F1),
            )

            fts = [[None] * RC for _ in range(NG)]

            def emit_pair(g, jj, osum_g, sden_g):
                """MM2 + numerator/denominator for j-chunks (2jj, 2jj+1)."""
                psh = psh_pool.tile([128, 2, GW], F32, tag="psh")
                for jp in range(2):
                    j = 2 * jj + jp
                    for rc in range(RC):
                        nc.tensor.matmul(
                            psh[:, jp, 0:NC2],
                            fts[g][rc][:, j * 128:(j + 1) * 128],
                            w_sb[:, rc, :],
                            start=(rc == 0),
                            stop=(rc == RC - 1),
                        )
                # GPSIMD cannot read PSUM, so everything touching psh is DVE.
                # bf16 scratch halves the reduce's input cost.
                xv = xc_sb[:, g, 2 * jj:2 * jj + 2, :]     # [128, 2, F1]
                xb = bass.AP(
                    tensor=xv.tensor,
                    offset=xv.offset,
                    ap=[xv.ap[0], xv.ap[1], [0, O], xv.ap[2]],
                )
                scratch = scr_pool.tile([128, 2, O, F1], BF16, tag="scr")
                nc.vector.tensor_mul(
                    scratch[:],
                    psh[:, :, 0:SCOL].rearrange("p j (o f) -> p j o f", o=O),
                    xb,
                )
                nc.vector.reduce_sum(
                    out=osum_g[:, jj], in_=scratch[:], axis=mybir.AxisListType.X
                )
                nc.vector.tensor_scalar_add(
                    sden_g[:, jj], psh[:, :, SCOL:SCOL + 1], 1e-8
                )

            def emit_mm2_combine(g):
                out_t = ot_pool.tile([128, JG, O], F32, tag="outt")
                osum_g = sm_pool.tile([128, 2, 2, O], F32, tag="osum")
                sden_g = sm_pool.tile([128, 2, 2, 1], F32, tag="sden")
                emit_pair(g, 0, osum_g, sden_g)
                emit_pair(g, 1, osum_g, sden_g)
                rec = sm_pool.tile([128, 2, 2, 1], F32, tag="rec")
                nc.vector.reciprocal(rec[:], sden_g[:])
                ra = rec[:]
                rb = bass.AP(
                    tensor=ra.tensor,
                    offset=ra.offset,
                    ap=[ra.ap[0], ra.ap[1], ra.ap[2], [0, O]],
                )
                # Final scale is SBUF-only, so GpSimd takes it off DVE.
                nc.gpsimd.tensor_mul(
                    out_t[:].rearrange("p (h t) o -> p h t o", h=2), osum_g[:], rb
                )
                nc.sync.dma_start(out=out_d[:, g], in_=out_t[:])

            for g in range(NG):
                for rc in range(RC):
                    ps1 = ps1_pool.tile([128, GW], F32, tag="ps1")
                    nc.tensor.matmul(
                        ps1[:],
                        xd_sb[:, 0, rc * 128:(rc + 1) * 128],
                        xd_sb[:, 1 + g, :],
                        start=True,
                        stop=True,
                    )
                    ft = ft_pool.tile([128, GW], F32R, tag="ft")
                    nc.scalar.activation(ft[:], ps1[:], EXP)
                    fts[g][rc] = ft
                # MM2 one group behind: its 4 exp tiles are already done.
                if g > 0:
                    emit_mm2_combine(g - 1)
            emit_mm2_combine(NG - 1)
    nc.compile()
    return nc


def get_program():
    global _PROGRAM
    if _PROGRAM is None:
        _PROGRAM = _build_program()
    return _PROGRAM


def _split_f16(arr):
    hi = arr.astype(np.float16)
    lo = (arr - hi.astype(np.float32)).astype(np.float16)
    return np.ascontiguousarray(hi), np.ascontiguousarray(lo)


def make_in_maps(inputs, centers, widths, consequent_w, consequent_b):
    x = np.ascontiguousarray(np.asarray(inputs, dtype=np.float32))
    c64 = np.asarray(centers, dtype=np.float64)
    w64 = np.asarray(widths, dtype=np.float64)
    W = np.asarray(consequent_w, dtype=np.float32)
    cb = np.asarray(consequent_b, dtype=np.float32)

    s = np.abs(w64) + 0.1
    a = -1.0 / (2.0 * s * s)                       # [R]

    daug = np.empty((K1, R), dtype=np.float64)
    daug[:F] = (c64 * (-2.0 * a)[:, None]).T       # -2*c[r,f]*a[r]
    daug[F] = (c64 * c64).sum(axis=1) * a          # c2[r]*a[r]  (pairs with ones)
    daug[F + 1] = a                                # a[r]        (pairs with x2)
    dh, dl = _split_f16(daug.astype(np.float32))
    dpack = np.concatenate([dh, dh, dl], axis=0)   # [102, R]

    # Waug columns: col o*33+f = W[r,f,o] (f<32), col o*33+32 = cb[r,o],
    # col 264 = 1.0 (S), cols 265.. = 0 (pad).
    wt = np.zeros((R, NC2P), dtype=np.float32)
    for o in range(O):
        wt[:, o * F1:o * F1 + F] = W[:, :, o]
        wt[:, o * F1 + F] = cb[:, o]
    wt[:, SCOL] = 1.0
    wt = np.ascontiguousarray(wt.reshape(RC, 128, NC2P).transpose(1, 0, 2))

    x2 = np.einsum("bf,bf->b", x, x).astype(np.float32)
    xat = np.empty((K1, B), dtype=np.float32)
    xat[:F] = x.T
    xat[F] = 1.0
    xat[F + 1] = x2
    xah, xal = _split_f16(xat)
    xpk = np.concatenate([xah, xal, xah], axis=0)  # [102, B]

    xaug = np.empty((B, F1), dtype=np.float32)
    xaug[:, :F] = x
    xaug[:, F] = 1.0

    in_maps = []
    for ci in range(NCORES):
        sl = slice(ci * BL, (ci + 1) * BL)
        xd = np.zeros((KP, NG + 1, GWP), dtype=np.float16)
        xd[:, 0, :GW] = dpack
        xd[:, 1:, :GW] = xpk[:, sl].reshape(KP, NG, GW)
        xc = np.zeros((128, NG, XCWP), dtype=np.float32)
        xc[:, :, :XCW] = (
            xaug[sl].reshape(NG, JG, 128, F1)
            .transpose(2, 0, 1, 3)
            .reshape(128, NG, XCW)
        )
        in_maps.append({
            "xd": np.ascontiguousarray(xd),
            "wt": wt,
            "xc": np.ascontiguousarray(xc),
        })
    return in_maps


def assemble_out(results):
    """[128, NG, JG*O] per core -> full [B, O] (b = g*512 + j*128 + p)."""
    outs = []
    for r in results:
        o = np.asarray(r["out"]).reshape(128, NG, JG, O)
        outs.append(o.transpose(1, 2, 0, 3).reshape(BL, O))
    return np.concatenate(outs, axis=0)


def _axon_reset():
    """Recover a wedged NeuronCore (NRT_EXEC_UNIT_UNRECOVERABLE) via the axon
    client's reset entry point.  Best-effort: silently skipped off-axon."""
    try:
        import ctypes
        import time

        import jax

        jax.devices()
        lib = ctypes.CDLL("/opt/axon/libaxon_pjrt.so")
        lib.axon_reset.restype = ctypes.c_int64
        lib.axon_reset()
        time.sleep(2)
    except Exception:
        pass


def kernel(inputs, centers, widths, consequent_w, consequent_b):
    nc = get_program()
    in_maps = make_in_maps(inputs, centers, widths, consequent_w, consequent_b)
    last_err = None
    for attempt in range(3):
        try:
            res = run_bass_kernel_spmd(nc, in_maps, list(range(NCORES))).results
            return assemble_out(res)
        except Exception as e:  # wedged device -> reset + retry
            last_err = e
            _axon_reset()
    raise last_err
